# revision 51
# baseline (speedup 1.0000x reference)
"""BandSimVQ Trainium2 kernel (8 NeuronCores, SPMD data-parallel over batch).

Reference computation (per batch b, band k):
    implicit[c,e] = sum_d codebooks[k,c,d] * W[k,d,e]          # [CS, D]
    d2[t,c]      = ||x[b,k,:,t] - implicit[c,:]||^2
    idx[t]       = argmin_c d2[t,c]
    q[e,t]       = implicit[idx[t], e]
    loss         = 1.25 * mean_{b,k,t} min_c d2[t,c]
Outputs: (quantized=[B,K,D,T] f32, indices=[B,K,T] i32, loss scalar f32).

Kernel strategy: core b handles batch b (data-parallel over B=8).
Phase 1: the implicit codebook transform (W @ cbT, shared across batches)
is sharded 8 ways: core r computes columns [256r, 256r+256) of
implicitT[e,c] for every band (fp32 PSUM), splits them into bf16 hi/lo,
and a per-band AllGather (implicit hi/lo + c2 slice packed in one buffer)
replicates the full implicitT to all cores.  AllGathers for later bands
overlap earlier bands' phase-2 compute.
Phase 2 per band: score[t,c] = x^T @ implicitT - c2/2 where the matmul
runs as a 3-pass bf16 split (xhi*ihi + xhi*ilo + xlo*ihi, fp32 PSUM
accumulate; abs error ~5e-6, well under near-tie flip threshold), the
c2/2 subtract is fused into the PSUM->SBUF drain on the vector engine,
and argmin uses the DVE max8/find_index8 instructions.  q[e,t] =
implicit[idx[t],:] is produced by a dma_gather (DMA-engine row gather
with 16-bit transpose) of bf16 codebook rows followed by a small bf16
W matmul, deferred by one band so the index round trip and gather
overlap the next band's scores.  The loss uses sum_t ||x_t||^2
(activation-accumulated squares) minus twice the score maxima,
reduced on-chip to one scalar per core and combined on the host.
"""

import numpy as np

B, NB, D, T = 8, 6, 512, 768     # batch, bands, feature dim, frames
CS, CD = 2048, 512               # codebook size, codebook dim
NCORES = 8

ETILES = D // 128                # 4  (e = output feature dim)
DTILES = CD // 128               # 4  (d = codebook dim, contraction)
TTILES = T // 128                # 6
CCHUNK = 256                     # c-chunk width for matmul free dim
NCCH = CS // CCHUNK              # 8
CSH = CS // NCORES               # 256  (c-columns per core in the shard)
AGN = ETILES * 128 * CSH + CSH   # per-rank allgather payload (impl slice + c2)


def _build_bass():
    import concourse.bass as bass
    import concourse.mybir as mybir
    from concourse import bacc
    from concourse.tile import TileContext

    f32 = mybir.dt.float32
    i32 = mybir.dt.int32
    i16 = mybir.dt.int16
    u32 = mybir.dt.uint32

    nc = bacc.Bacc(None, target_bir_lowering=False, debug=False)

    x_ext = nc.declare_dram_parameter("x", [NB, D, T], f32, isOutput=False)
    cbT_ext = nc.declare_dram_parameter("cbT", [NB, CD, CS], f32, isOutput=False)
    w_ext = nc.declare_dram_parameter("w", [NB, CD, D], f32, isOutput=False)
    bf16 = mybir.dt.bfloat16
    cbbf_ext = nc.declare_dram_parameter("cb_bf", [NB, CS, CD], bf16, isOutput=False)
    wbf_ext = nc.declare_dram_parameter("w_bf", [NB, CD, D], bf16, isOutput=False)
    outq_ext = nc.declare_dram_parameter("out_q", [NB, D, T], f32, isOutput=True)
    outi_ext = nc.declare_dram_parameter("out_idx", [NB, T], i32, isOutput=True)
    outp_ext = nc.declare_dram_parameter("out_partial", [1, 1], f32, isOutput=True)

    idx_scr = nc.dram_tensor("idx_scr", [NB, T], i16)
    agin = [nc.dram_tensor(f"agin{k}", [AGN], f32) for k in range(NB)]
    agout = [nc.dram_tensor(f"agout{k}", [NCORES * AGN], f32, addr_space="Shared")
             for k in range(NB)]

    with TileContext(nc) as tc:
        with (
            tc.tile_pool(name="weights", bufs=1) as wpool,
            tc.tile_pool(name="cbtsl", bufs=2) as cbtpool,
            tc.tile_pool(name="xband", bufs=2) as xpool,
            tc.tile_pool(name="implt", bufs=2) as iplpool,
            tc.tile_pool(name="score", bufs=2) as scpool,
            tc.tile_pool(name="small", bufs=3) as smpool,
            tc.tile_pool(name="const", bufs=1) as cpool,
            tc.tile_pool(name="c2p", bufs=2) as c2pool,
            tc.tile_pool(name="qout", bufs=2) as qpool,
            tc.tile_pool(name="psum_i", bufs=2, space="PSUM") as ppool_i,
            tc.tile_pool(name="psum_c2", bufs=1, space="PSUM") as ppool_c2,
            tc.tile_pool(name="psum_s", bufs=2, space="PSUM") as ppool_s,
            tc.tile_pool(name="psum_fin", bufs=1, space="PSUM") as ppool_fin,
        ):
            ones_sb = cpool.tile([128, 1], f32, tag="ones")
            nc.vector.memset(ones_sb[:], 1.0)

            NACC = NB * ETILES + NB * TTILES          # 24 + 36 = 60
            acc_all = cpool.tile([128, NACC], f32, tag="acc")
            nc.vector.memset(acc_all[:], 0.0)

            pid = nc.sync.partition_id()
            RHALF = 0.7071067811865476

            # ============ phase 1: implicit shard + per-band AllGather ======
            def load_w(k):
                w_sb = [wpool.tile([128, D], f32, tag=f"w{di}", name=f"w_sb{k}_{di}")
                        for di in range(DTILES)]
                for di in range(DTILES):
                    nc.sync.dma_start(w_sb[di][:], w_ext[k, 128 * di:128 * (di + 1), :])
                cb_sb = [cbtpool.tile([128, CSH], f32, tag=f"cb{di}", name=f"cb_sb{k}_{di}")
                         for di in range(DTILES)]
                for di in range(DTILES):
                    nc.sync.dma_start(
                        cb_sb[di][:],
                        cbT_ext[k, 128 * di:128 * (di + 1), bass.ds(pid * CSH, CSH)],
                    )
                return w_sb, cb_sb

            wcur = load_w(0)
            for k in range(NB):
                w_sb, cb_sb = wcur
                if k + 1 < NB:
                    wcur = load_w(k + 1)
                ihi = scpool.tile([128, ETILES * CSH], bf16, tag="ihi", name=f"ihi{k}")
                ilo = scpool.tile([128, ETILES * CSH], bf16, tag="ilo", name=f"ilo{k}")
                c2_ps = ppool_c2.tile([1, CSH], f32)
                for ei in range(ETILES):
                    esl = bass.ts(ei, 128)
                    impl_ps = ppool_i.tile([128, CSH], f32, tag="ips", name=f"impl_ps{k}_{ei}")
                    for di in range(DTILES):
                        nc.tensor.matmul(
                            impl_ps[:], w_sb[di][:, esl], cb_sb[di][:],
                            start=(di == 0), stop=(di == DTILES - 1),
                        )
                    # hi = bf16(impl), lo = bf16(impl - hi)
                    nc.vector.tensor_copy(ihi[:, bass.ts(ei, CSH)], impl_ps[:])
                    nc.vector.scalar_tensor_tensor(
                        ilo[:, bass.ts(ei, CSH)], impl_ps[:], 1.0,
                        ihi[:, bass.ts(ei, CSH)],
                        op0=mybir.AluOpType.mult, op1=mybir.AluOpType.subtract,
                    )
                    sq_sb = smpool.tile([128, CSH], f32, tag="sq", name=f"sq{k}_{ei}")
                    nc.scalar.activation(
                        sq_sb[:], impl_ps[:],
                        mybir.ActivationFunctionType.Square, scale=RHALF,
                    )
                    nc.tensor.matmul(
                        c2_ps[:], ones_sb[:], sq_sb[:],
                        start=(ei == 0), stop=(ei == ETILES - 1),
                    )
                c2sl_sb = smpool.tile([1, CSH], f32, tag="c2sl", name=f"c2sl{k}")
                nc.vector.tensor_copy(c2sl_sb[:], c2_ps[:])
                # pack [impl hi | impl lo | c2 slice] into the allgather input
                agi = agin[k].ap()
                agib = agi[0:2 * ETILES * 128 * CSH // 2].bitcast(bf16)
                nc.sync.dma_start(
                    agib[0:ETILES * 128 * CSH].rearrange(
                        "(e p c) -> p e c", p=128, c=CSH), ihi[:],
                )
                nc.sync.dma_start(
                    agib[ETILES * 128 * CSH:2 * ETILES * 128 * CSH].rearrange(
                        "(e p c) -> p e c", p=128, c=CSH), ilo[:],
                )
                nc.sync.dma_start(agi[ETILES * 128 * CSH:], c2sl_sb[:])
                nc.gpsimd.collective_compute(
                    "AllGather", mybir.AluOpType.bypass,
                    replica_groups=[list(range(NCORES))],
                    ins=[agin[k].ap().opt()],
                    outs=[agout[k].ap().opt()],
                )

            # ============ phase 2: scores / argmin / gather per band ========
            def load_x(k):
                x_f = []
                for ei in range(ETILES):
                    x_sb = xpool.tile([128, T], f32, tag=f"xf{ei}", name=f"x_sb{k}_{ei}")
                    nc.sync.dma_start(x_sb[:], x_ext[k, 128 * ei:128 * (ei + 1), :])
                    xsq_sb = xpool.tile([128, T], f32, tag="xsq", name=f"xsq{k}_{ei}")
                    nc.scalar.activation(
                        xsq_sb[:], x_sb[:],
                        mybir.ActivationFunctionType.Square,
                        accum_out=acc_all[:, k * ETILES + ei: k * ETILES + ei + 1],
                    )
                    x_f.append(x_sb)
                wbf_sb = [xpool.tile([128, D], bf16, tag=f"wbf{di}", name=f"wbf_sb{k}_{di}")
                          for di in range(DTILES)]
                for di in range(DTILES):
                    nc.sync.dma_start(wbf_sb[di][:], wbf_ext[k, 128 * di:128 * (di + 1), :])
                return x_f, wbf_sb

            def split_x(k, x_f):
                xhi = [xpool.tile([128, T], bf16, tag=f"xh{ei}", name=f"xh{k}_{ei}")
                       for ei in range(ETILES)]
                xlo = [xpool.tile([128, T], bf16, tag=f"xl{ei}", name=f"xl{k}_{ei}")
                       for ei in range(ETILES)]
                for ei in range(ETILES):
                    nc.vector.tensor_copy(xhi[ei][:], x_f[ei][:])
                    nc.vector.scalar_tensor_tensor(
                        xlo[ei][:], x_f[ei][:], 1.0, xhi[ei][:],
                        op0=mybir.AluOpType.mult, op1=mybir.AluOpType.subtract,
                    )
                return xhi, xlo

            NSEG = ETILES * 128 * CSH

            def prep_band(k):
                # unpack gathered implicitT (bf16 hi/lo) and c2 for band k
                ago = agout[k].ap().rearrange("(r n) -> r n", r=NCORES)
                agob = agout[k].ap().bitcast(bf16).rearrange("(r n) -> r n", r=NCORES)
                ipl_hi = [iplpool.tile([128, CS], bf16, tag=f"iph{ei}",
                                       name=f"iph{k}_{ei}") for ei in range(ETILES)]
                ipl_lo = [iplpool.tile([128, CS], bf16, tag=f"ipo{ei}",
                                       name=f"ipo{k}_{ei}") for ei in range(ETILES)]
                for ei in range(ETILES):
                    srch = agob[:, ei * 128 * CSH:(ei + 1) * 128 * CSH].rearrange(
                        "r (p c) -> p r c", p=128, c=CSH)
                    nc.sync.dma_start(ipl_hi[ei][:], srch)
                    srcl = agob[:, NSEG + ei * 128 * CSH:NSEG + (ei + 1) * 128 * CSH].rearrange(
                        "r (p c) -> p r c", p=128, c=CSH)
                    nc.sync.dma_start(ipl_lo[ei][:], srcl)
                c2rep = c2pool.tile([128, CS], f32, tag="c2rep", name=f"c2rep{k}")
                nc.sync.dma_start(c2rep[:], ago[:, NSEG:].partition_broadcast(128))
                return ipl_hi, ipl_lo, c2rep

            TCH = 384

            def emit_q(k, cbgT_sb, wbf_sb, on_dve=False):
                for ei in range(ETILES):
                    gq_sb = qpool.tile([128, T], f32, tag="gq", name=f"gq_sb{k}_{ei}")
                    for tc_ in range(T // TCH):
                        q_ps = ppool_i.tile([128, TCH], f32, tag="ips",
                                            name=f"q_ps{k}_{ei}_{tc_}")
                        for di in range(DTILES):
                            nc.tensor.matmul(
                                q_ps[:], wbf_sb[di][:, bass.ts(ei, 128)],
                                cbgT_sb[:, di, bass.ds(tc_ * TCH, TCH)],
                                start=(di == 0), stop=(di == DTILES - 1),
                            )
                        if on_dve:
                            nc.vector.tensor_copy(gq_sb[:, bass.ts(tc_, TCH)], q_ps[:])
                        else:
                            nc.scalar.copy(gq_sb[:, bass.ts(tc_, TCH)], q_ps[:])
                    nc.scalar.dma_start(outq_ext[k, 128 * ei:128 * (ei + 1), :], gq_sb[:])

            qprev = None
            xcur = load_x(0)
            scur = split_x(0, xcur[0])
            pcur = prep_band(0)
            for k in range(NB):
                x_f, wbf_sb = xcur
                xhi, xlo = scur
                ipl_hi, ipl_lo, c2rep = pcur
                if k + 1 < NB:
                    xcur = load_x(k + 1)
                    pcur = prep_band(k + 1)

                # ---- score, argmax ----
                idx16_sb = smpool.tile([128, TTILES], i16, tag="idx16", name=f"idx16_{k}")
                for tt in range(TTILES):
                    tsl = bass.ts(tt, 128)
                    score_sb = scpool.tile([128, CS], f32, tag="score", name=f"score{k}_{tt}")
                    for cjp in range(CS // 1024):
                        sc_ps = ppool_s.tile([128, 1024], f32)
                        for hf in range(2):
                            csl = bass.ds(cjp * 1024 + hf * 512, 512)
                            psl = bass.ts(hf, 512)
                            for ei in range(ETILES):
                                nc.tensor.matmul(
                                    sc_ps[:, psl], xhi[ei][:, tsl], ipl_hi[ei][:, csl],
                                    start=(ei == 0), stop=False,
                                )
                            for ei in range(ETILES):
                                last = ei == ETILES - 1
                                nc.tensor.matmul(
                                    sc_ps[:, psl], xhi[ei][:, tsl], ipl_lo[ei][:, csl],
                                    start=False, stop=False,
                                )
                                nc.tensor.matmul(
                                    sc_ps[:, psl], xlo[ei][:, tsl], ipl_hi[ei][:, csl],
                                    start=False, stop=last,
                                )
                        nc.vector.tensor_sub(
                            score_sb[:, bass.ts(cjp, 1024)], sc_ps[:],
                            c2rep[:, bass.ts(cjp, 1024)],
                        )
                    mx8 = smpool.tile([128, 8], f32, tag="mx8", name=f"mx8_{k}_{tt}")
                    ix8 = smpool.tile([128, 8], u32, tag="ix8", name=f"ix8_{k}_{tt}")
                    nc.vector.max(mx8[:], score_sb[:])
                    nc.vector.max_index(ix8[:], mx8[:], score_sb[:])
                    nc.gpsimd.dma_start(
                        outi_ext[k, 128 * tt:128 * (tt + 1)], ix8[:, 0:1].bitcast(i32)
                    )
                    nc.vector.tensor_copy(idx16_sb[:, tt:tt + 1], ix8[:, 0:1])
                    nc.vector.tensor_scalar_mul(
                        acc_all[:, NB * ETILES + k * TTILES + tt: NB * ETILES + k * TTILES + tt + 1],
                        mx8[:, 0:1], -2.0,
                    )

                if k + 1 < NB:
                    scur = split_x(k + 1, xcur[0])
                if qprev is not None:
                    emit_q(*qprev)
                # ---- index layout round trip (p-major write, wrapped readback) ----
                nc.gpsimd.dma_start(
                    idx_scr[k].rearrange("(p c) -> p c", c=TTILES), idx16_sb[:]
                )
                idxw_sb = smpool.tile([128, T // 16], i16, tag="idxw", name=f"idxw{k}")
                wr_view = idx_scr[k].rearrange(
                    "(ph r c) -> r c ph", ph=8, r=16, c=TTILES)
                for g in range(8):
                    nc.gpsimd.dma_start(
                        idxw_sb[16 * g:16 * (g + 1), :].rearrange(
                            "r (c ph) -> r c ph", ph=8),
                        wr_view,
                    )
                cbgT_sb = qpool.tile([128, DTILES, T], bf16, tag="cbgT", name=f"cbgT{k}")
                nc.gpsimd.dma_gather(
                    cbgT_sb[:], cbbf_ext[k], idxw_sb[:],
                    num_idxs=T, num_idxs_reg=T, elem_size=CD, transpose=True,
                )
                qprev = (k, cbgT_sb, wbf_sb)
            emit_q(*qprev, on_dve=True)

            # ---- final loss partial: sum over accumulator ----
            fin_ps = ppool_fin.tile([1, NACC], f32)
            nc.tensor.matmul(fin_ps[:], ones_sb[:], acc_all[:], start=True, stop=True)
            fin_sb = smpool.tile([1, NACC], f32, tag="fin")
            nc.vector.tensor_copy(fin_sb[:], fin_ps[:])
            part_sb = smpool.tile([1, 1], f32, tag="part")
            nc.vector.tensor_reduce(
                part_sb[:], fin_sb[:], mybir.AxisListType.X, mybir.AluOpType.add
            )
            nc.sync.dma_start(outp_ext[:], part_sb[:])

    nc.finalize()
    return nc


_NC = None


def make_in_maps(x, codebooks, W):
    import ml_dtypes

    x = np.ascontiguousarray(x, dtype=np.float32)
    cbT = np.ascontiguousarray(codebooks.transpose(0, 2, 1), dtype=np.float32)
    w = np.ascontiguousarray(W, dtype=np.float32)
    cb_bf = np.ascontiguousarray(codebooks.astype(ml_dtypes.bfloat16))
    w_bf = np.ascontiguousarray(W.astype(ml_dtypes.bfloat16))
    return [{"x": x[b], "cbT": cbT, "w": w, "cb_bf": cb_bf, "w_bf": w_bf}
            for b in range(NCORES)]


def kernel(x, codebooks, W):
    from concourse.bass_utils import run_bass_kernel_spmd

    global _NC
    if _NC is None:
        _NC = _build_bass()

    in_maps = make_in_maps(x, codebooks, W)
    res = run_bass_kernel_spmd(_NC, in_maps, list(range(NCORES)))

    quant = np.stack([res.results[b]["out_q"] for b in range(NCORES)])
    idx = np.stack([res.results[b]["out_idx"] for b in range(NCORES)])
    partial = sum(float(res.results[b]["out_partial"][0, 0]) for b in range(NCORES))
    loss = np.float32(1.25 * partial / (B * NB * T))
    return quant, idx.astype(np.int32), loss


# revision 52
# speedup vs baseline: 1.0808x; 1.0808x over previous
"""BandSimVQ Trainium2 kernel (8 NeuronCores, SPMD data-parallel over batch).

Reference computation (per batch b, band k):
    implicit[c,e] = sum_d codebooks[k,c,d] * W[k,d,e]          # [CS, D]
    d2[t,c]      = ||x[b,k,:,t] - implicit[c,:]||^2
    idx[t]       = argmin_c d2[t,c]
    q[e,t]       = implicit[idx[t], e]
    loss         = 1.25 * mean_{b,k,t} min_c d2[t,c]
Outputs: (quantized=[B,K,D,T] f32, indices=[B,K,T] i32, loss scalar f32).

Kernel strategy: core b handles batch b (data-parallel over B=8).
Phase 1: the implicit codebook transform (W @ cbT, shared across batches)
is sharded 8 ways: core r computes columns [256r, 256r+256) of
implicitT[e,c] for every band (fp32 PSUM), splits them into bf16 hi/lo,
and a per-band AllGather (implicit hi/lo + c2 slice packed in one buffer)
replicates the full implicitT to all cores.  AllGathers for later bands
overlap earlier bands' phase-2 compute.
Phase 2 per band: score[t,c] = x^T @ implicitT - c2/2 where the matmul
runs as a 3-pass bf16 split (xhi*ihi + xhi*ilo + xlo*ihi, fp32 PSUM
accumulate; abs error ~5e-6, well under near-tie flip threshold), the
c2/2 subtract is fused into the PSUM->SBUF drain on the vector engine,
and argmin uses the DVE max8/find_index8 instructions.  q[e,t] =
implicit[idx[t],:] is produced by a dma_gather (DMA-engine row gather
with 16-bit transpose) of bf16 codebook rows followed by a small bf16
W matmul, deferred by one band so the index round trip and gather
overlap the next band's scores.  The loss uses sum_t ||x_t||^2
(activation-accumulated squares) minus twice the score maxima,
reduced on-chip to one scalar per core and combined on the host.
"""

import numpy as np

B, NB, D, T = 8, 6, 512, 768     # batch, bands, feature dim, frames
CS, CD = 2048, 512               # codebook size, codebook dim
NCORES = 8

ETILES = D // 128                # 4  (e = output feature dim)
DTILES = CD // 128               # 4  (d = codebook dim, contraction)
TTILES = T // 128                # 6
CCHUNK = 256                     # c-chunk width for matmul free dim
NCCH = CS // CCHUNK              # 8
CSH = CS // NCORES               # 256  (c-columns per core in the shard)
AGN = ETILES * 128 * CSH + CSH   # per-rank allgather payload (impl slice + c2)


def _build_bass():
    import concourse.bass as bass
    import concourse.mybir as mybir
    from concourse import bacc
    from concourse.tile import TileContext

    f32 = mybir.dt.float32
    i32 = mybir.dt.int32
    i16 = mybir.dt.int16
    u32 = mybir.dt.uint32

    nc = bacc.Bacc(None, target_bir_lowering=False, debug=False)

    x_ext = nc.declare_dram_parameter("x", [NB, D, T], f32, isOutput=False)
    cbT_ext = nc.declare_dram_parameter("cbT", [NB, CD, CS], f32, isOutput=False)
    w_ext = nc.declare_dram_parameter("w", [NB, CD, D], f32, isOutput=False)
    bf16 = mybir.dt.bfloat16
    cbbf_ext = nc.declare_dram_parameter("cb_bf", [NB, CS, CD], bf16, isOutput=False)
    wbf_ext = nc.declare_dram_parameter("w_bf", [NB, CD, D], bf16, isOutput=False)
    outq_ext = nc.declare_dram_parameter("out_q", [NB, D, T], f32, isOutput=True)
    outi_ext = nc.declare_dram_parameter("out_idx", [NB, T], i32, isOutput=True)
    outp_ext = nc.declare_dram_parameter("out_partial", [1, 1], f32, isOutput=True)

    idx_scr = nc.dram_tensor("idx_scr", [NB, T], i16)
    agin = [nc.dram_tensor(f"agin{k}", [AGN], f32) for k in range(NB)]
    agout = [nc.dram_tensor(f"agout{k}", [NCORES * AGN], f32, addr_space="Shared")
             for k in range(NB)]

    with TileContext(nc) as tc:
        with (
            tc.tile_pool(name="weights", bufs=1) as wpool,
            tc.tile_pool(name="cbtsl", bufs=1) as cbtpool,
            tc.tile_pool(name="xband", bufs=2) as xpool,
            tc.tile_pool(name="implt", bufs=2) as iplpool,
            tc.tile_pool(name="score", bufs=2) as scpool,
            tc.tile_pool(name="small", bufs=3) as smpool,
            tc.tile_pool(name="const", bufs=1) as cpool,
            tc.tile_pool(name="c2p", bufs=2) as c2pool,
            tc.tile_pool(name="qout", bufs=2) as qpool,
            tc.tile_pool(name="psum_i", bufs=2, space="PSUM") as ppool_i,
            tc.tile_pool(name="psum_c2", bufs=1, space="PSUM") as ppool_c2,
            tc.tile_pool(name="psum_s", bufs=2, space="PSUM") as ppool_s,
            tc.tile_pool(name="psum_fin", bufs=1, space="PSUM") as ppool_fin,
        ):
            ones_sb = cpool.tile([128, 1], f32, tag="ones")
            nc.vector.memset(ones_sb[:], 1.0)

            NACC = NB * ETILES + NB * TTILES          # 24 + 36 = 60
            acc_all = cpool.tile([128, NACC], f32, tag="acc")
            nc.vector.memset(acc_all[:], 0.0)

            pid = nc.sync.partition_id()
            RHALF = 0.7071067811865476

            # ============ phase 1: implicit shard + per-band AllGather ======
            def load_w(k):
                w_sb = [wpool.tile([128, D], f32, tag=f"w{di}", name=f"w_sb{k}_{di}")
                        for di in range(DTILES)]
                for di in range(DTILES):
                    nc.sync.dma_start(w_sb[di][:], w_ext[k, 128 * di:128 * (di + 1), :])
                cb_sb = [cbtpool.tile([128, CSH], f32, tag=f"cb{di}", name=f"cb_sb{k}_{di}")
                         for di in range(DTILES)]
                for di in range(DTILES):
                    nc.sync.dma_start(
                        cb_sb[di][:],
                        cbT_ext[k, 128 * di:128 * (di + 1), bass.ds(pid * CSH, CSH)],
                    )
                return w_sb, cb_sb

            wcur = load_w(0)
            for k in range(NB):
                w_sb, cb_sb = wcur
                if k + 1 < NB:
                    wcur = load_w(k + 1)
                ihi = scpool.tile([128, ETILES * CSH], bf16, tag="ihi", name=f"ihi{k}")
                ilo = scpool.tile([128, ETILES * CSH], bf16, tag="ilo", name=f"ilo{k}")
                c2_ps = ppool_c2.tile([1, CSH], f32)
                for ei in range(ETILES):
                    esl = bass.ts(ei, 128)
                    impl_ps = ppool_i.tile([128, CSH], f32, tag="ips", name=f"impl_ps{k}_{ei}")
                    for di in range(DTILES):
                        nc.tensor.matmul(
                            impl_ps[:], w_sb[di][:, esl], cb_sb[di][:],
                            start=(di == 0), stop=(di == DTILES - 1),
                        )
                    # hi = bf16(impl), lo = bf16(impl - hi)
                    nc.vector.tensor_copy(ihi[:, bass.ts(ei, CSH)], impl_ps[:])
                    nc.vector.scalar_tensor_tensor(
                        ilo[:, bass.ts(ei, CSH)], impl_ps[:], 1.0,
                        ihi[:, bass.ts(ei, CSH)],
                        op0=mybir.AluOpType.mult, op1=mybir.AluOpType.subtract,
                    )
                    sq_sb = smpool.tile([128, CSH], f32, tag="sq", name=f"sq{k}_{ei}")
                    nc.scalar.activation(
                        sq_sb[:], impl_ps[:],
                        mybir.ActivationFunctionType.Square, scale=RHALF,
                    )
                    nc.tensor.matmul(
                        c2_ps[:], ones_sb[:], sq_sb[:],
                        start=(ei == 0), stop=(ei == ETILES - 1),
                    )
                c2sl_sb = smpool.tile([1, CSH], f32, tag="c2sl", name=f"c2sl{k}")
                nc.vector.tensor_copy(c2sl_sb[:], c2_ps[:])
                # pack [impl hi | impl lo | c2 slice] into the allgather input
                agi = agin[k].ap()
                agib = agi[0:2 * ETILES * 128 * CSH // 2].bitcast(bf16)
                nc.sync.dma_start(
                    agib[0:ETILES * 128 * CSH].rearrange(
                        "(e p c) -> p e c", p=128, c=CSH), ihi[:],
                )
                nc.sync.dma_start(
                    agib[ETILES * 128 * CSH:2 * ETILES * 128 * CSH].rearrange(
                        "(e p c) -> p e c", p=128, c=CSH), ilo[:],
                )
                nc.sync.dma_start(agi[ETILES * 128 * CSH:], c2sl_sb[:])
                nc.gpsimd.collective_compute(
                    "AllGather", mybir.AluOpType.bypass,
                    replica_groups=[list(range(NCORES))],
                    ins=[agin[k].ap().opt()],
                    outs=[agout[k].ap().opt()],
                )

            # ============ phase 2: scores / argmin / gather per band ========
            def load_x(k):
                x_f = []
                for ei in range(ETILES):
                    x_sb = xpool.tile([128, T], f32, tag=f"xf{ei}", name=f"x_sb{k}_{ei}")
                    nc.sync.dma_start(x_sb[:], x_ext[k, 128 * ei:128 * (ei + 1), :])
                    xsq_sb = xpool.tile([128, T], f32, tag="xsq", name=f"xsq{k}_{ei}")
                    nc.scalar.activation(
                        xsq_sb[:], x_sb[:],
                        mybir.ActivationFunctionType.Square,
                        accum_out=acc_all[:, k * ETILES + ei: k * ETILES + ei + 1],
                    )
                    x_f.append(x_sb)
                wbf_sb = [xpool.tile([128, D], bf16, tag=f"wbf{di}", name=f"wbf_sb{k}_{di}")
                          for di in range(DTILES)]
                for di in range(DTILES):
                    nc.sync.dma_start(wbf_sb[di][:], wbf_ext[k, 128 * di:128 * (di + 1), :])
                return x_f, wbf_sb

            def split_x(k, x_f):
                xhi = [xpool.tile([128, T], bf16, tag=f"xh{ei}", name=f"xh{k}_{ei}")
                       for ei in range(ETILES)]
                xlo = [xpool.tile([128, T], bf16, tag=f"xl{ei}", name=f"xl{k}_{ei}")
                       for ei in range(ETILES)]
                for ei in range(ETILES):
                    nc.vector.tensor_copy(xhi[ei][:], x_f[ei][:])
                    nc.vector.scalar_tensor_tensor(
                        xlo[ei][:], x_f[ei][:], 1.0, xhi[ei][:],
                        op0=mybir.AluOpType.mult, op1=mybir.AluOpType.subtract,
                    )
                return xhi, xlo

            NSEG = ETILES * 128 * CSH

            def prep_band(k):
                # unpack gathered implicitT (bf16 hi/lo) and c2 for band k
                ago = agout[k].ap().rearrange("(r n) -> r n", r=NCORES)
                agob = agout[k].ap().bitcast(bf16).rearrange("(r n) -> r n", r=NCORES)
                ipl_hi = [iplpool.tile([128, CS], bf16, tag=f"iph{ei}",
                                       name=f"iph{k}_{ei}") for ei in range(ETILES)]
                ipl_lo = [iplpool.tile([128, CS], bf16, tag=f"ipo{ei}",
                                       name=f"ipo{k}_{ei}") for ei in range(ETILES)]
                for ei in range(ETILES):
                    srch = agob[:, ei * 128 * CSH:(ei + 1) * 128 * CSH].rearrange(
                        "r (p c) -> p r c", p=128, c=CSH)
                    nc.sync.dma_start(ipl_hi[ei][:], srch)
                    srcl = agob[:, NSEG + ei * 128 * CSH:NSEG + (ei + 1) * 128 * CSH].rearrange(
                        "r (p c) -> p r c", p=128, c=CSH)
                    nc.sync.dma_start(ipl_lo[ei][:], srcl)
                c2rep = c2pool.tile([128, CS], f32, tag="c2rep", name=f"c2rep{k}")
                nc.sync.dma_start(c2rep[:], ago[:, NSEG:].partition_broadcast(128))
                return ipl_hi, ipl_lo, c2rep

            TCH = 384

            def emit_q(k, cbgT_sb, wbf_sb, on_dve=False):
                for ei in range(ETILES):
                    gq_sb = qpool.tile([128, T], f32, tag="gq", name=f"gq_sb{k}_{ei}")
                    for tc_ in range(T // TCH):
                        q_ps = ppool_i.tile([128, TCH], f32, tag="ips",
                                            name=f"q_ps{k}_{ei}_{tc_}")
                        for di in range(DTILES):
                            nc.tensor.matmul(
                                q_ps[:], wbf_sb[di][:, bass.ts(ei, 128)],
                                cbgT_sb[:, di, bass.ds(tc_ * TCH, TCH)],
                                start=(di == 0), stop=(di == DTILES - 1),
                            )
                        if on_dve:
                            nc.vector.tensor_copy(gq_sb[:, bass.ts(tc_, TCH)], q_ps[:])
                        else:
                            nc.scalar.copy(gq_sb[:, bass.ts(tc_, TCH)], q_ps[:])
                    nc.scalar.dma_start(outq_ext[k, 128 * ei:128 * (ei + 1), :], gq_sb[:])

            qprev = None
            xcur = load_x(0)
            scur = split_x(0, xcur[0])
            pcur = prep_band(0)
            for k in range(NB):
                x_f, wbf_sb = xcur
                xhi, xlo = scur
                ipl_hi, ipl_lo, c2rep = pcur
                if k + 1 < NB:
                    xcur = load_x(k + 1)
                    pcur = prep_band(k + 1)

                # ---- score, argmax ----
                idx16_sb = smpool.tile([128, TTILES], i16, tag="idx16", name=f"idx16_{k}")
                for tt in range(TTILES):
                    tsl = bass.ts(tt, 128)
                    score_sb = scpool.tile([128, CS], f32, tag="score", name=f"score{k}_{tt}")
                    for cjp in range(CS // 1024):
                        sc_ps = ppool_s.tile([128, 1024], f32)
                        for hf in range(2):
                            csl = bass.ds(cjp * 1024 + hf * 512, 512)
                            psl = bass.ts(hf, 512)
                            for ei in range(ETILES):
                                nc.tensor.matmul(
                                    sc_ps[:, psl], xhi[ei][:, tsl], ipl_hi[ei][:, csl],
                                    start=(ei == 0), stop=False,
                                )
                            for ei in range(ETILES):
                                last = ei == ETILES - 1
                                nc.tensor.matmul(
                                    sc_ps[:, psl], xhi[ei][:, tsl], ipl_lo[ei][:, csl],
                                    start=False, stop=False,
                                )
                                nc.tensor.matmul(
                                    sc_ps[:, psl], xlo[ei][:, tsl], ipl_hi[ei][:, csl],
                                    start=False, stop=last,
                                )
                        nc.vector.tensor_sub(
                            score_sb[:, bass.ts(cjp, 1024)], sc_ps[:],
                            c2rep[:, bass.ts(cjp, 1024)],
                        )
                    mx8 = smpool.tile([128, 8], f32, tag="mx8", name=f"mx8_{k}_{tt}")
                    ix8 = smpool.tile([128, 8], u32, tag="ix8", name=f"ix8_{k}_{tt}")
                    nc.vector.max(mx8[:], score_sb[:])
                    nc.vector.max_index(ix8[:], mx8[:], score_sb[:])
                    nc.gpsimd.dma_start(
                        outi_ext[k, 128 * tt:128 * (tt + 1)], ix8[:, 0:1].bitcast(i32)
                    )
                    nc.vector.tensor_copy(idx16_sb[:, tt:tt + 1], ix8[:, 0:1])
                    nc.vector.tensor_scalar_mul(
                        acc_all[:, NB * ETILES + k * TTILES + tt: NB * ETILES + k * TTILES + tt + 1],
                        mx8[:, 0:1], -2.0,
                    )

                if k + 1 < NB:
                    scur = split_x(k + 1, xcur[0])
                if qprev is not None:
                    emit_q(*qprev)
                # ---- index layout round trip (p-major write, wrapped readback) ----
                nc.gpsimd.dma_start(
                    idx_scr[k].rearrange("(p c) -> p c", c=TTILES), idx16_sb[:]
                )
                idxw_sb = smpool.tile([128, T // 16], i16, tag="idxw", name=f"idxw{k}")
                wr_view = idx_scr[k].rearrange(
                    "(ph r c) -> r c ph", ph=8, r=16, c=TTILES)
                for g in range(8):
                    nc.gpsimd.dma_start(
                        idxw_sb[16 * g:16 * (g + 1), :].rearrange(
                            "r (c ph) -> r c ph", ph=8),
                        wr_view,
                    )
                cbgT_sb = qpool.tile([128, DTILES, T], bf16, tag="cbgT", name=f"cbgT{k}")
                nc.gpsimd.dma_gather(
                    cbgT_sb[:], cbbf_ext[k], idxw_sb[:],
                    num_idxs=T, num_idxs_reg=T, elem_size=CD, transpose=True,
                )
                qprev = (k, cbgT_sb, wbf_sb)
            emit_q(*qprev, on_dve=True)

            # ---- final loss partial: sum over accumulator ----
            fin_ps = ppool_fin.tile([1, NACC], f32)
            nc.tensor.matmul(fin_ps[:], ones_sb[:], acc_all[:], start=True, stop=True)
            fin_sb = smpool.tile([1, NACC], f32, tag="fin")
            nc.vector.tensor_copy(fin_sb[:], fin_ps[:])
            part_sb = smpool.tile([1, 1], f32, tag="part")
            nc.vector.tensor_reduce(
                part_sb[:], fin_sb[:], mybir.AxisListType.X, mybir.AluOpType.add
            )
            nc.sync.dma_start(outp_ext[:], part_sb[:])

    nc.finalize()
    return nc


_NC = None


def make_in_maps(x, codebooks, W):
    import ml_dtypes

    x = np.ascontiguousarray(x, dtype=np.float32)
    cbT = np.ascontiguousarray(codebooks.transpose(0, 2, 1), dtype=np.float32)
    w = np.ascontiguousarray(W, dtype=np.float32)
    cb_bf = np.ascontiguousarray(codebooks.astype(ml_dtypes.bfloat16))
    w_bf = np.ascontiguousarray(W.astype(ml_dtypes.bfloat16))
    return [{"x": x[b], "cbT": cbT, "w": w, "cb_bf": cb_bf, "w_bf": w_bf}
            for b in range(NCORES)]


def kernel(x, codebooks, W):
    from concourse.bass_utils import run_bass_kernel_spmd

    global _NC
    if _NC is None:
        _NC = _build_bass()

    in_maps = make_in_maps(x, codebooks, W)
    res = run_bass_kernel_spmd(_NC, in_maps, list(range(NCORES)))

    quant = np.stack([res.results[b]["out_q"] for b in range(NCORES)])
    idx = np.stack([res.results[b]["out_idx"] for b in range(NCORES)])
    partial = sum(float(res.results[b]["out_partial"][0, 0]) for b in range(NCORES))
    loss = np.float32(1.25 * partial / (B * NB * T))
    return quant, idx.astype(np.int32), loss
